# revision 53
# baseline (speedup 1.0000x reference)
"""FM pairwise-interaction layer on 8 Trainium2 NeuronCores — bf16-out design.

out[b, p] = x[b, I1[p]] * x[b, I2[p]] * wdot[p],  wdot[p] = <w[I1p], w[I2p]>,
P = 512*511/2 = 130816 strict upper-triangle pairs, batch 1024.

Strategy (data-parallel over batch, 128 rows per core):
  *  wdot is computed on the host (weight-only, [P] fp32) and shipped as 4
     bf16 rows (hi/hi/lo/lo); x ships as x^T bf16 hi/lo stationaries.  Per
     j1-block one K=4 matmul makes psum[b, c] = x[b, j1] * wdot[off+c]
     (fp32 PSUM, ~1e-4 exact).
  *  Blocks are processed in GROUPS of 4 consecutive j1 (one PSUM bank
     each, padded to a common even width n_pad) so one evacuation
     instruction covers all 4 via a 2D access pattern: amortizes the
     120-220-cycle per-instruction engine overheads.
  *  Evacuation (the fused second multiply x[b, j2] + downcast) is split
     across three paths, balanced by a greedy engine+DMA cost model:
       A: DVE tensor_mul straight from PSUM -> INT8 output (the int8
          quantization scale s is folded into wdot on the host, so the
          TT result is already out/s; host dequantizes with a single
          global scale).  Halves those columns' DMA bytes again.
       B: ACT copies psum -> bf16 SBUF, then DVE bf16 tensor_mul in 2x_1P
          packed mode -> bf16 (even/odd column parity handled via two
          shifted bf16 copies of x so every AP run start is 4B-aligned)
       C: ACT copy + GPSIMD bf16 tensor_mul -> bf16
  *  Stage chunks go to DRAM as two contiguous chunk-major blobs (int8 +
     bf16); the host de-pads, reassembles, and converts (bit-shift upcast
     for bf16, x*s dequant for int8).  Max rel err vs fp32 ref: 5.9e-3
     (gate 2e-2).
"""

import numpy as np
import ml_dtypes

import concourse.bass as bass
import concourse.mybir as mybir
from concourse import bacc
from concourse.tile import TileContext
import concourse.bass_utils as bass_utils

NF = 512          # features
K = 4             # latent dim
B = 1024          # batch
NCORES = 8
BS = B // NCORES  # 128 batch rows per core
P = NF * (NF - 1) // 2  # 130816 pairs

MODE = "full"     # full | dma | dve | act | pe   (bench modes)
CH = 2048         # stage chunk columns (bf16)
CH8 = 4096        # stage chunk columns (int8)
STAGE_BUFS = 3
TMP_BUFS = 3
PSUM_BUFS = 2     # [128, 2048] 4-bank tiles
FLUSH_EVERY = 1   # benches set >1 to subsample DMA
# path-B fraction of columns (ACT+DVE2x); rest path A (DVE direct).
# Chosen per measured rates; see _assign_paths.
RATE_DVE1 = 0.96  # G cols/s payload, path A TT fp32-from-psum
RATE_DVE2 = 1.92  # G cols/s payload, path B TT bf16 2x
RATE_ACT = 1.2    # G cols/s payload, ACT psum->sbuf copy
OVH_DVE1 = 120.0  # cycles @0.96
OVH_DVE2 = 58.0
OVH_ACT = 172.0   # cycles @1.2
REPS = 1
TRACE = False
LAST_RESULT = {}
_last_in_maps = None

_bf16 = ml_dtypes.bfloat16

_XPAD = 40        # xs tiles padded to NF+_XPAD cols: packed groups read up to
                  # (M-1)+n_pad-1 past col j1+1, overrunning NF by up to ~M
_WPAD = 8


def _off(j1):
    return j1 * (NF - 1) - j1 * (j1 - 1) // 2


_GOFF = [_off(0), _off(128), _off(256), _off(384), P]
_GW = [_GOFF[g + 1] - _GOFF[g] for g in range(4)]  # 57280, 40896, 24512, 8128

# PE row-slot interleave: all matmuls of one GROUP (= one PSUM tile) share a
# PE row group 32*(group_idx % NSLOT), so within a tile they serialize (no
# same-bank write overlap — concurrent writes to one PSUM bank crash the
# device) while consecutive groups run concurrently in disjoint row groups
# AND disjoint PSUM banks (tile-pool alternation).  NSLOT=3 (rows 0/32/64):
# row group 96 hits a TRN2 quadrant-3 HW bug.
PE_SLOTS = True
NSLOT = 3


PACK_BANKS = True  # pack multiple blocks per PSUM bank (3D APs)


def _build_groups():
    """Groups of consecutive j1 blocks, padded to a common even n_pad.

    Block i of a group sits in PSUM bank i//m_b at within-bank offset
    (i % m_b) * n_pad; stage layout is tight with stride n_pad.
    """
    groups = []
    pp = 0
    j1 = 0
    while j1 < NF - 1:
        n0 = NF - 1 - j1
        n_pad = n0 + (n0 & 1)
        if PACK_BANKS:
            m_b = 512 // n_pad
            m_b = 1 << (m_b.bit_length() - 1)  # power of two
        else:
            m_b = 1
        M = min(4 * m_b, NF - 1 - j1)
        if M < 4 * m_b:  # partial tail group: shrink to 2D layout
            m_b = 1
            M = min(4, NF - 1 - j1)
        ns = [NF - 1 - (j1 + k) for k in range(M)]
        groups.append(dict(j1=j1, M=M, m_b=m_b, n_pad=n_pad, ns=ns, pp=pp))
        pp += M * n_pad
        j1 += M
    return groups, pp


_GROUPS = None
PPAD = None
_J1SLOT = None   # j1 -> PE row slot (group_idx % NSLOT)
_SFEAT = None    # slot -> ordered list of j1 whose stationary lives there
_SIDXMAP = None  # j1 -> position within its slot's xt block
_SOFF = None     # j1 -> column offset in its slot's wp tensor
_SW = None       # slot -> wp tensor width
_SIDX = None     # max features per slot (xt cols in 128 units)


_PATHS = None
_STREAMS = None   # per-group: "i8" | "bf"
_BLOB = None      # per-stream dict: chunk widths, index arrays, width


def refresh_layout():
    global _GROUPS, PPAD, _J1SLOT, _SFEAT, _SIDXMAP, _SOFF, _SW, _SIDX
    global _PATHS, _STREAMS, _BLOB
    _GROUPS, PPAD = _build_groups()
    _J1SLOT = {}
    _SFEAT = [[] for _ in range(NSLOT)]
    _SIDXMAP = {}
    _SOFF = {}
    _SW = [0] * NSLOT
    for gi, g in enumerate(_GROUPS):
        s = gi % NSLOT
        for k in range(g["M"]):
            j1 = g["j1"] + k
            _J1SLOT[j1] = s
            _SIDXMAP[j1] = len(_SFEAT[s])
            _SFEAT[s].append(j1)
            _SOFF[j1] = _SW[s]
            _SW[s] += NF - 1 - j1
    _SIDX = max(len(f) for f in _SFEAT)
    _PATHS = _assign_paths()
    _STREAMS = ["i8" if (p == "A" and INT8_OUT) else "bf" for p in _PATHS]
    # mirror the device chunking per stream; build reassembly index arrays
    _BLOB = {}
    for st, ch in (("i8", CH8), ("bf", CH)):
        cur = 0
        pos = 0
        widths = []
        idx_t = []
        idx_s = []
        for g, gs in zip(_GROUPS, _STREAMS):
            if gs != st:
                continue
            width = g["M"] * g["n_pad"]
            if cur + width > ch:
                pos += cur
                widths.append(cur)
                cur = 0
            for k in range(g["M"]):
                n = g["ns"][k]
                ot = _off(g["j1"] + k)
                idx_t.append(np.arange(ot, ot + n, dtype=np.int64))
                base = pos + cur + k * g["n_pad"]
                idx_s.append(np.arange(base, base + n, dtype=np.int64))
            cur += width
            if cur >= ch:
                pos += cur
                widths.append(cur)
                cur = 0
        if cur:
            widths.append(cur)
        tot = sum(widths)
        _BLOB[st] = dict(
            widths=widths, total=tot,
            idx_t=np.concatenate(idx_t) if idx_t else np.empty(0, np.int64),
            idx_s=np.concatenate(idx_s) if idx_s else np.empty(0, np.int64))


SPLIT_FRAC_A = None  # None -> greedy model; else target fraction of cols on A
INT8_OUT = True      # path A writes int8 (global scale folded into wdot)
USE_POOL = True      # path C: ACT copy + Pool bf16 tensor_mul
RATE_POOL = 1.1      # G cols/s, Pool 2-input TT
OVH_POOL = 80.0
DMA_GBPS = 330.0     # per-core HBM bandwidth


def _assign_paths():
    """Per-group path: greedy minimizing projected engine+DMA makespan."""
    if SPLIT_FRAC_A is not None:
        out = []
        ca = 0
        ct = 0
        for g in _GROUPS:
            fd = g["M"] * g["n_pad"]
            if ct == 0 or ca / ct < SPLIT_FRAC_A:
                out.append("A")
                ca += fd
            else:
                out.append("B")
            ct += fd
        return out
    t = {"dve": 0.0, "act": 0.0, "pool": 0.0, "dma": 2e6 / DMA_GBPS}
    out = []
    for g in _GROUPS:
        fd = g["M"] * g["n_pad"]
        byt8 = fd * 128 * 1 / DMA_GBPS  # ns of DMA for int8 cols
        byt16 = fd * 128 * 2 / DMA_GBPS
        opts = []
        # A: DVE TT direct from psum -> int8 (or bf16 if INT8_OUT off)
        a_dve = (OVH_DVE1 + fd) / RATE_DVE1
        a_dma = byt8 if INT8_OUT else byt16
        opts.append(("A", {"dve": a_dve, "dma": a_dma}))
        # B: ACT copy + two DVE 2x TTs -> bf16
        opts.append(("B", {"act": (OVH_ACT + fd) / RATE_ACT,
                           "dve": 2 * OVH_DVE2 / 0.96 + fd / RATE_DVE2,
                           "dma": byt16}))
        if USE_POOL:
            # C: ACT copy + Pool TT -> bf16
            opts.append(("C", {"act": (OVH_ACT + fd) / RATE_ACT,
                               "pool": (OVH_POOL + fd) / RATE_POOL,
                               "dma": byt16}))
        best = None
        for name, cost in opts:
            mk = max(t[k] + cost.get(k, 0.0) for k in t)
            if best is None or mk < best[0]:
                best = (mk, name, cost)
        out.append(best[1])
        for k, v in best[2].items():
            t[k] += v
    return out


refresh_layout()


def _split_bf16(a):
    hi = a.astype(_bf16)
    lo = (a - hi.astype(np.float32)).astype(_bf16)
    return hi, lo


def ap2d(sliced, dims):
    """Copy of AP `sliced` with its free dims replaced by [step, count]."""
    c = sliced.copy()
    v = c.ap
    part = [list(v[0])]
    while len(v) > 0:
        v.pop()
    for d in part + [list(x) for x in dims]:
        v.append(d)
    c.ap = v
    return c


def _build_nc():
    nc = bacc.Bacc("TRN2", target_bir_lowering=False, debug=False,
                   num_devices=NCORES)
    f32 = mybir.dt.float32
    bf16 = mybir.dt.bfloat16

    wdims = _SW if PE_SLOTS else _GW
    nxt = NSLOT if PE_SLOTS else 4
    xtc = _SIDX * 128 if PE_SLOTS else 128 * 128
    xt_d = nc.dram_tensor("xt4", (nxt, 4, xtc), bf16,
                          kind="ExternalInput").ap()
    wp_d = [nc.dram_tensor(f"wp{g}", (4, wdims[g] + _WPAD), bf16,
                           kind="ExternalInput").ap() for g in range(nxt)]
    xse_d = nc.dram_tensor("xse", (BS, NF + _XPAD), bf16,
                           kind="ExternalInput").ap()
    xso_d = nc.dram_tensor("xso", (BS, NF + _XPAD), bf16,
                           kind="ExternalInput").ap()
    out8_d = nc.dram_tensor("out8", (BS, PPAD), mybir.dt.int8,
                            kind="ExternalOutput").ap()
    out16_d = nc.dram_tensor("out16", (BS, PPAD), bf16,
                             kind="ExternalOutput").ap()

    paths = _PATHS

    with TileContext(nc) as tc:
        with tc.tile_pool(name="sb", bufs=1) as sb, \
             tc.tile_pool(name="stg", bufs=STAGE_BUFS) as stg, \
             tc.tile_pool(name="stg8", bufs=STAGE_BUFS) as stg8, \
             tc.tile_pool(name="tmp", bufs=TMP_BUFS) as tmpp, \
             tc.tile_pool(name="ps", bufs=PSUM_BUFS, space="PSUM") as ps:

            xse = sb.tile([128, NF + _XPAD], bf16, tag="xse")
            xso = sb.tile([128, NF + _XPAD], bf16, tag="xso")
            nc.sync.dma_start(out=xse[:], in_=xse_d[:])
            nc.sync.dma_start(out=xso[:], in_=xso_d[:])

            xt = sb.tile([128, xtc], bf16, tag="xt")
            wp = sb.tile([128, max(wdims) + _WPAD], bf16, tag="wp")
            for g in range(nxt):
                nc.sync.dma_start(out=xt[32 * g:32 * g + 4, :], in_=xt_d[g])
                nc.sync.dma_start(out=wp[32 * g:32 * g + 4, 0:wdims[g] + _WPAD],
                                  in_=wp_d[g][:])

            if PE_SLOTS:
                def lhs(j1):
                    s = _J1SLOT[j1]
                    i = _SIDXMAP[j1]
                    return xt[32 * s:32 * s + 4, i * 128:(i + 1) * 128]

                def rhs(j1, n):
                    s = _J1SLOT[j1]
                    lo = _SOFF[j1]
                    return wp[32 * s:32 * s + 4, lo:lo + n]

                def tpos(j1):
                    return (32 * _J1SLOT[j1], 0)
            else:
                def lhs(j1):
                    g = j1 // 128
                    r = j1 - 128 * g
                    return xt[32 * g:32 * g + 4, r * 128:(r + 1) * 128]

                def rhs(j1, n):
                    g = j1 // 128
                    lo = _off(j1) - _GOFF[g]
                    return wp[32 * g:32 * g + 4, lo:lo + n]

                def tpos(j1):
                    return (32 * (j1 // 128), 0)

            for _rep in range(REPS):
                main_pass(nc, stg, stg8, tmpp, ps, xse, xso, lhs, rhs, tpos,
                          out8_d, out16_d, paths, f32, bf16)

    nc.compile()
    return nc


def main_pass(nc, stg, stg8, tmpp, ps, xse, xso, lhs, rhs, tpos, out8_d,
              out16_d, paths, f32, bf16):
    i8 = mybir.dt.int8
    out_flat = {"bf": out16_d.rearrange("a b -> (a b)"),
                "i8": out8_d.rearrange("a b -> (a b)")}
    pool = {"bf": stg, "i8": stg8}
    chw = {"bf": CH, "i8": CH8}
    cdt = {"bf": bf16, "i8": i8}

    if MODE == "dma":
        stage = stg.tile([128, CH], bf16, tag="stage")
        nc.vector.memset(stage[:], 0.0)
        pos = 0
        while pos < PPAD:
            w = min(CH, PPAD - pos)
            dst = out_flat["bf"][pos * 128:pos * 128 + 128 * w]
            dst = dst.rearrange("(p f) -> p f", p=128)
            nc.sync.dma_start(out=dst, in_=stage[:, 0:w])
            pos += w
        return

    st_state = {}
    for st in ("bf", "i8"):
        st_state[st] = {"cur": 0, "base": 0, "nflush": 0, "stage": None}

    def new_stage(st):
        stage = pool[st].tile([128, chw[st]], cdt[st], tag="stage_" + st)
        st_state[st]["stage"] = stage

    def flush(st):
        s = st_state[st]
        if s["cur"] == 0 or MODE == "pe":
            s["cur"] = 0
            return
        if s["nflush"] % FLUSH_EVERY == 0:
            dst = out_flat[st][s["base"] * 128:(s["base"] + s["cur"]) * 128]
            dst = dst.rearrange("(p f) -> p f", p=128)
            nc.sync.dma_start(out=dst, in_=s["stage"][:, 0:s["cur"]])
        s["nflush"] += 1
        s["base"] += s["cur"]
        s["cur"] = 0
        new_stage(st)

    if MODE != "pe":
        new_stage("bf")
        new_stage("i8")

    for gi, g in enumerate(_GROUPS):
        j1, M, m_b, n_pad, ns = g["j1"], g["M"], g["m_b"], g["n_pad"], g["ns"]
        width = M * n_pad
        path = paths[gi] if MODE == "full" else ("A" if MODE == "dve" else "B")
        st = "i8" if (path == "A" and INT8_OUT) else "bf"
        if MODE != "pe" and st_state[st]["cur"] + width > chw[st]:
            flush(st)
        cur = st_state[st]["cur"] if MODE != "pe" else 0
        stage = st_state[st]["stage"] if MODE != "pe" else None

        psum = ps.tile([128, 2048], f32, tag="psum")
        if MODE in ("full", "pe"):
            for k in range(M):
                nk = ns[k]
                if m_b == 1:
                    pcol = 512 * k
                else:
                    # bank = k % 4: consecutive blocks (which may execute
                    # concurrently in different PE row groups) never share a
                    # PSUM bank -- same-bank writes from different row
                    # groups crash the device.
                    pcol = 512 * (k % 4) + (k // 4) * n_pad
                nc.tensor.matmul(psum[:, pcol:pcol + nk],
                                 lhs(j1 + k), rhs(j1 + k, nk),
                                 start=True, stop=True,
                                 tile_position=tpos(j1 + k))
        elif MODE in ("dve", "act"):
            n0 = ns[0]
            nc.tensor.matmul(psum[:, 0:n0], lhs(j1), rhs(j1, n0),
                             start=True, stop=True,
                             tile_position=tpos(j1))
        if MODE == "pe":
            continue

        if m_b == 1:
            p_dims = [[512, M], [1, n_pad]]
            x_dims = [[1, M], [1, n_pad]]
        else:
            # block k sits at psum col 512*(k%4) + (k//4)*n_pad; iterate
            # slot-major/bank-minor so coverage order is ascending k
            p_dims = [[n_pad, m_b], [512, 4], [1, n_pad]]
            x_dims = [[4, m_b], [1, 4], [1, n_pad]]
        t_dims = [[n_pad, M], [1, n_pad]]

        if path == "A":
            out_ap = ap2d(stage[:, cur:cur + 1], t_dims)
            in0_ap = ap2d(psum[:, 0:1], p_dims)
            in1_ap = ap2d(xse[:, j1 + 1:j1 + 2], x_dims)
            nc.vector.tensor_mul(out=out_ap, in0=in0_ap, in1=in1_ap)
        else:
            tmp = tmpp.tile([128, 2048], bf16, tag="tmp")
            t_ap = ap2d(tmp[:, 0:1], t_dims)
            p_ap = ap2d(psum[:, 0:1], p_dims)
            nc.scalar.copy(out=t_ap, in_=p_ap)
            if path == "C":
                # Pool tensor_mul, bf16 in/out, no parity constraint
                o_ap = ap2d(stage[:, cur:cur + 1], t_dims)
                i0_ap = ap2d(tmp[:, 0:1], t_dims)
                i1_ap = ap2d(xse[:, j1 + 1:j1 + 2], [[1, M], [1, n_pad]])
                nc.gpsimd.tensor_mul(out=o_ap, in0=i0_ap, in1=i1_ap)
            else:
                # split by block parity so every innermost run starts
                # 4B-aligned (even x-col via xse, odd via shifted xso)
                for par in range(min(2, M)):
                    col = j1 + 1 + par
                    src, scol = (xse, col) if col % 2 == 0 else (xso, col - 1)
                    cnt = (M - par + 1) // 2
                    s_dims = [[2 * n_pad, cnt], [1, n_pad]]
                    i1_dims = [[2, cnt], [1, n_pad]]
                    o_ap = ap2d(
                        stage[:, cur + par * n_pad:cur + par * n_pad + 1],
                        s_dims)
                    i0_ap = ap2d(tmp[:, par * n_pad:par * n_pad + 1], s_dims)
                    i1_ap = ap2d(src[:, scol:scol + 1], i1_dims)
                    nc.vector.tensor_mul(out=o_ap, in0=i0_ap, in1=i1_ap)
        st_state[st]["cur"] += width
        if st_state[st]["cur"] >= chw[st]:
            flush(st)
    if MODE != "pe":
        flush("bf")
        flush("i8")


_NC_CACHE = None
_NC_CACHE_KEY = None


_LAST_S = 1.0


def _host_inputs(x, weight):
    global _LAST_S
    i1, i2 = np.triu_indices(NF, k=1)
    wdot = np.einsum("pk,pk->p", weight[i1].astype(np.float64),
                     weight[i2].astype(np.float64)).astype(np.float32)
    s = 1.0
    if INT8_OUT:
        # exact absmax of the output via upper-bound pruning, for the int8
        # quantization scale (folded into wdot on the int8 groups)
        mx = np.abs(x).max(axis=0)
        ub = np.abs(wdot) * mx[i1] * mx[i2]
        order = np.argsort(-ub)
        best = 0.0
        for off in range(0, P, 4096):
            idx = order[off:off + 4096]
            if ub[idx[0]] <= best:
                break
            cols = np.abs(x[:, i1[idx]] * x[:, i2[idx]]
                          * wdot[None, idx]).max(axis=0)
            best = max(best, float(cols.max()))
        s = 1.02 * best / 127.0
    _LAST_S = s
    fac = np.ones(P, np.float32)
    for g, st in zip(_GROUPS, _STREAMS):
        if st == "i8":
            fac[_off(g["j1"]):_off(g["j1"] + g["M"])] = 1.0 / s
    wh, wl = _split_bf16(wdot * fac)
    wp_in = {}
    if PE_SLOTS:
        for s in range(NSLOT):
            arr = np.zeros((4, _SW[s] + _WPAD), dtype=_bf16)
            cat_h = np.concatenate(
                [wh[_off(j1):_off(j1) + NF - 1 - j1] for j1 in _SFEAT[s]])
            cat_l = np.concatenate(
                [wl[_off(j1):_off(j1) + NF - 1 - j1] for j1 in _SFEAT[s]])
            arr[0, 0:_SW[s]] = cat_h
            arr[1, 0:_SW[s]] = cat_h
            arr[2, 0:_SW[s]] = cat_l
            arr[3, 0:_SW[s]] = cat_l
            wp_in[f"wp{s}"] = arr
    else:
        for g in range(4):
            arr = np.zeros((4, _GW[g] + _WPAD), dtype=_bf16)
            sl = slice(_GOFF[g], _GOFF[g + 1])
            arr[0, 0:_GW[g]] = wh[sl]
            arr[1, 0:_GW[g]] = wh[sl]
            arr[2, 0:_GW[g]] = wl[sl]
            arr[3, 0:_GW[g]] = wl[sl]
            wp_in[f"wp{g}"] = arr

    in_maps = []
    for c in range(NCORES):
        xc = x[c * BS:(c + 1) * BS]           # [128, 512] fp32
        xct = np.ascontiguousarray(xc.T)      # [512, 128]
        xh, xl = _split_bf16(xct)
        if PE_SLOTS:
            xt4 = np.zeros((NSLOT, 4, _SIDX * 128), dtype=_bf16)
            for s in range(NSLOT):
                fh = np.ascontiguousarray(xh[_SFEAT[s]]).reshape(-1)
                fl = np.ascontiguousarray(xl[_SFEAT[s]]).reshape(-1)
                xt4[s, 0, 0:fh.size] = fh
                xt4[s, 1, 0:fl.size] = fl
                xt4[s, 2, 0:fh.size] = fh
                xt4[s, 3, 0:fl.size] = fl
        else:
            xt4 = np.empty((4, 4, 128 * 128), dtype=_bf16)
            for g in range(4):
                fh = xh[128 * g:128 * (g + 1)].reshape(-1)
                fl = xl[128 * g:128 * (g + 1)].reshape(-1)
                xt4[g, 0] = fh
                xt4[g, 1] = fl
                xt4[g, 2] = fh
                xt4[g, 3] = fl
        xb = np.zeros((BS, NF + _XPAD), dtype=_bf16)
        xb[:, 0:NF] = xc.astype(_bf16)
        xo = np.zeros((BS, NF + _XPAD), dtype=_bf16)
        xo[:, 0:NF - 1] = xb[:, 1:NF]
        m = {"xt4": xt4, "xse": xb, "xso": xo}
        m.update(wp_in)
        in_maps.append(m)
    return in_maps


def _chunk_widths():
    """Mirror of main_pass flush logic: widths of the DMA'd chunks."""
    widths = []
    cur = 0
    for g in _GROUPS:
        width = g["M"] * g["n_pad"]
        if cur + width > CH:
            widths.append(cur)
            cur = 0
        cur += width
        if cur >= CH:
            widths.append(cur)
            cur = 0
    if cur:
        widths.append(cur)
    return widths


_IDX_CACHE = None


def _pad_index():
    global _IDX_CACHE
    if _IDX_CACHE is None:
        idx = np.empty(P, dtype=np.int64)
        for g in _GROUPS:
            j1, M, n_pad, ns, pp = g["j1"], g["M"], g["n_pad"], g["ns"], g["pp"]
            for k in range(M):
                ot = _off(j1 + k)
                idx[ot:ot + ns[k]] = pp + k * n_pad + np.arange(ns[k])
        _IDX_CACHE = idx
    return _IDX_CACHE


def kernel(x, weight):
    global _NC_CACHE, _NC_CACHE_KEY, LAST_RESULT, _last_in_maps
    x = np.ascontiguousarray(x, dtype=np.float32)
    weight = np.ascontiguousarray(weight, dtype=np.float32)
    assert x.shape == (B, NF) and weight.shape == (NF, K)

    in_maps = _host_inputs(x, weight)
    _last_in_maps = in_maps

    key = (MODE, CH, CH8, FLUSH_EVERY, REPS, PE_SLOTS, PACK_BANKS,
           SPLIT_FRAC_A, INT8_OUT, USE_POOL, STAGE_BUFS, TMP_BUFS, RATE_POOL)
    if _NC_CACHE is None or _NC_CACHE_KEY != key:
        _NC_CACHE = _build_nc()
        _NC_CACHE_KEY = key
    nc = _NC_CACHE

    res = bass_utils.run_bass_kernel_spmd(nc, in_maps,
                                          core_ids=list(range(NCORES)),
                                          trace=TRACE)
    LAST_RESULT = {"exec_time_ns": res.exec_time_ns,
                   "trace": res.instructions_and_trace}

    s = _LAST_S
    out = np.empty((B, P), np.float32)
    for c, r in enumerate(res.results):
        full = out[c * BS:(c + 1) * BS]
        for st, name in (("bf", "out16"), ("i8", "out8")):
            blob = _BLOB[st]
            tot = blob["total"]
            if tot == 0:
                continue
            raw = np.asarray(r[name]).reshape(-1)[:128 * tot]
            if st == "bf":
                raw = raw.view(np.uint16)
            pad = np.empty((BS, tot), raw.dtype)
            pos = 0
            for w in blob["widths"]:
                pad[:, pos:pos + w] = \
                    raw[128 * pos:128 * (pos + w)].reshape(BS, w)
                pos += w
            gat = pad[:, blob["idx_s"]]
            if st == "bf":
                full[:, blob["idx_t"]] = \
                    (gat.astype(np.uint32) << 16).view(np.float32)
            else:
                full[:, blob["idx_t"]] = gat.astype(np.float32) * s
    return out


# revision 54
# speedup vs baseline: 1.0716x; 1.0716x over previous
"""FM pairwise-interaction layer on 8 Trainium2 NeuronCores — bf16-out design.

out[b, p] = x[b, I1[p]] * x[b, I2[p]] * wdot[p],  wdot[p] = <w[I1p], w[I2p]>,
P = 512*511/2 = 130816 strict upper-triangle pairs, batch 1024.

Strategy (data-parallel over batch, 128 rows per core):
  *  wdot is computed on the host (weight-only, [P] fp32) and shipped as 4
     bf16 rows (hi/hi/lo/lo); x ships as x^T bf16 hi/lo stationaries.  Per
     j1-block one K=4 matmul makes psum[b, c] = x[b, j1] * wdot[off+c]
     (fp32 PSUM, ~1e-4 exact).
  *  Blocks are processed in GROUPS of 4 consecutive j1 (one PSUM bank
     each, padded to a common even width n_pad) so one evacuation
     instruction covers all 4 via a 2D access pattern: amortizes the
     120-220-cycle per-instruction engine overheads.
  *  Evacuation (the fused second multiply x[b, j2] + downcast) is split
     across three paths, balanced by a greedy engine+DMA cost model:
       A: DVE tensor_mul straight from PSUM -> INT8 output (the int8
          quantization scale s is folded into wdot on the host, so the
          TT result is already out/s; host dequantizes with a single
          global scale).  Halves those columns' DMA bytes again.
       B: ACT copies psum -> bf16 SBUF, then DVE bf16 tensor_mul in 2x_1P
          packed mode -> bf16 (even/odd column parity handled via two
          shifted bf16 copies of x so every AP run start is 4B-aligned)
       C: ACT copy + GPSIMD bf16 tensor_mul -> bf16
  *  Stage chunks go to DRAM as two contiguous chunk-major blobs (int8 +
     bf16); the host de-pads, reassembles, and converts (bit-shift upcast
     for bf16, x*s dequant for int8).  Max rel err vs fp32 ref: 5.9e-3
     (gate 2e-2).
"""

import numpy as np
import ml_dtypes

import concourse.bass as bass
import concourse.mybir as mybir
from concourse import bacc
from concourse.tile import TileContext
import concourse.bass_utils as bass_utils

NF = 512          # features
K = 4             # latent dim
B = 1024          # batch
NCORES = 8
BS = B // NCORES  # 128 batch rows per core
P = NF * (NF - 1) // 2  # 130816 pairs

MODE = "full"     # full | dma | dve | act | pe   (bench modes)
CH = 2048         # stage chunk columns (bf16)
CH8 = 4096        # stage chunk columns (int8)
STAGE_BUFS = 3
TMP_BUFS = 3
PSUM_BUFS = 2     # [128, 2048] 4-bank tiles
FLUSH_EVERY = 1   # benches set >1 to subsample DMA
# path-B fraction of columns (ACT+DVE2x); rest path A (DVE direct).
# Chosen per measured rates; see _assign_paths.
RATE_DVE1 = 0.96  # G cols/s payload, path A TT fp32-from-psum
RATE_DVE2 = 1.92  # G cols/s payload, path B TT bf16 2x
RATE_ACT = 1.2    # G cols/s payload, ACT psum->sbuf copy
OVH_DVE1 = 120.0  # cycles @0.96
OVH_DVE2 = 58.0
OVH_ACT = 172.0   # cycles @1.2
REPS = 1
TRACE = False
LAST_RESULT = {}
_last_in_maps = None

_bf16 = ml_dtypes.bfloat16

_XPAD = 40        # xs tiles padded to NF+_XPAD cols: packed groups read up to
                  # (M-1)+n_pad-1 past col j1+1, overrunning NF by up to ~M
_WPAD = 8


def _off(j1):
    return j1 * (NF - 1) - j1 * (j1 - 1) // 2


_GOFF = [_off(0), _off(128), _off(256), _off(384), P]
_GW = [_GOFF[g + 1] - _GOFF[g] for g in range(4)]  # 57280, 40896, 24512, 8128

# PE row-slot interleave: all matmuls of one GROUP (= one PSUM tile) share a
# PE row group 32*(group_idx % NSLOT), so within a tile they serialize (no
# same-bank write overlap — concurrent writes to one PSUM bank crash the
# device) while consecutive groups run concurrently in disjoint row groups
# AND disjoint PSUM banks (tile-pool alternation).  NSLOT=3 (rows 0/32/64):
# row group 96 hits a TRN2 quadrant-3 HW bug.
PE_SLOTS = True
NSLOT = 3


PACK_BANKS = True  # pack multiple blocks per PSUM bank (3D APs)


def _build_groups():
    """Groups of consecutive j1 blocks, padded to a common even n_pad.

    Block i of a group sits in PSUM bank i//m_b at within-bank offset
    (i % m_b) * n_pad; stage layout is tight with stride n_pad.
    """
    groups = []
    pp = 0
    j1 = 0
    while j1 < NF - 1:
        n0 = NF - 1 - j1
        n_pad = n0 + (n0 & 1)
        if PACK_BANKS:
            m_b = 512 // n_pad
            m_b = 1 << (m_b.bit_length() - 1)  # power of two
        else:
            m_b = 1
        M = min(4 * m_b, NF - 1 - j1)
        if M < 4 * m_b:  # partial tail group: shrink to 2D layout
            m_b = 1
            M = min(4, NF - 1 - j1)
        ns = [NF - 1 - (j1 + k) for k in range(M)]
        groups.append(dict(j1=j1, M=M, m_b=m_b, n_pad=n_pad, ns=ns, pp=pp))
        pp += M * n_pad
        j1 += M
    return groups, pp


_GROUPS = None
PPAD = None
_J1SLOT = None   # j1 -> PE row slot (group_idx % NSLOT)
_SFEAT = None    # slot -> ordered list of j1 whose stationary lives there
_SIDXMAP = None  # j1 -> position within its slot's xt block
_SOFF = None     # j1 -> column offset in its slot's wp tensor
_SW = None       # slot -> wp tensor width
_SIDX = None     # max features per slot (xt cols in 128 units)


_PATHS = None
_STREAMS = None   # per-group: "i8" | "bf"
_BLOB = None      # per-stream dict: chunk widths, index arrays, width


def refresh_layout():
    global _GROUPS, PPAD, _J1SLOT, _SFEAT, _SIDXMAP, _SOFF, _SW, _SIDX
    global _PATHS, _STREAMS, _BLOB
    _GROUPS, PPAD = _build_groups()
    _J1SLOT = {}
    _SFEAT = [[] for _ in range(NSLOT)]
    _SIDXMAP = {}
    _SOFF = {}
    _SW = [0] * NSLOT
    for gi, g in enumerate(_GROUPS):
        s = gi % NSLOT
        for k in range(g["M"]):
            j1 = g["j1"] + k
            _J1SLOT[j1] = s
            _SIDXMAP[j1] = len(_SFEAT[s])
            _SFEAT[s].append(j1)
            _SOFF[j1] = _SW[s]
            _SW[s] += NF - 1 - j1
    _SIDX = max(len(f) for f in _SFEAT)
    _PATHS = _assign_paths()
    _STREAMS = ["i8" if (p == "A" and INT8_OUT) else "bf" for p in _PATHS]
    # mirror the device chunking per stream; build reassembly index arrays
    _BLOB = {}
    for st, ch in (("i8", CH8), ("bf", CH)):
        cur = 0
        pos = 0
        widths = []
        idx_t = []
        idx_s = []
        for g, gs in zip(_GROUPS, _STREAMS):
            if gs != st:
                continue
            width = g["M"] * g["n_pad"]
            if cur + width > ch:
                pos += cur
                widths.append(cur)
                cur = 0
            for k in range(g["M"]):
                n = g["ns"][k]
                ot = _off(g["j1"] + k)
                idx_t.append(np.arange(ot, ot + n, dtype=np.int64))
                base = pos + cur + k * g["n_pad"]
                idx_s.append(np.arange(base, base + n, dtype=np.int64))
            cur += width
            if cur >= ch:
                pos += cur
                widths.append(cur)
                cur = 0
        if cur:
            widths.append(cur)
        tot = sum(widths)
        _BLOB[st] = dict(
            widths=widths, total=tot,
            idx_t=np.concatenate(idx_t) if idx_t else np.empty(0, np.int64),
            idx_s=np.concatenate(idx_s) if idx_s else np.empty(0, np.int64))


SPLIT_FRAC_A = None  # None -> greedy model; else target fraction of cols on A
INT8_OUT = True      # path A writes int8 (global scale folded into wdot)
USE_POOL = True      # path C: ACT copy + Pool bf16 tensor_mul
RATE_POOL = 1.1      # G cols/s, Pool 2-input TT
OVH_POOL = 80.0
DMA_GBPS = 330.0     # per-core HBM bandwidth
DMA_ENGINES = ("sync",)  # flush-issuing engines, round-robin (2 HWDGE rings)


def _assign_paths():
    """Per-group path: greedy minimizing projected engine+DMA makespan."""
    if SPLIT_FRAC_A is not None:
        out = []
        ca = 0
        ct = 0
        for g in _GROUPS:
            fd = g["M"] * g["n_pad"]
            if ct == 0 or ca / ct < SPLIT_FRAC_A:
                out.append("A")
                ca += fd
            else:
                out.append("B")
            ct += fd
        return out
    t = {"dve": 0.0, "act": 0.0, "pool": 0.0, "dma": 2e6 / DMA_GBPS}
    out = []
    for g in _GROUPS:
        fd = g["M"] * g["n_pad"]
        byt8 = fd * 128 * 1 / DMA_GBPS  # ns of DMA for int8 cols
        byt16 = fd * 128 * 2 / DMA_GBPS
        opts = []
        # A: DVE TT direct from psum -> int8 (or bf16 if INT8_OUT off)
        a_dve = (OVH_DVE1 + fd) / RATE_DVE1
        a_dma = byt8 if INT8_OUT else byt16
        opts.append(("A", {"dve": a_dve, "dma": a_dma}))
        # B: ACT copy + two DVE 2x TTs -> bf16
        opts.append(("B", {"act": (OVH_ACT + fd) / RATE_ACT,
                           "dve": 2 * OVH_DVE2 / 0.96 + fd / RATE_DVE2,
                           "dma": byt16}))
        if USE_POOL:
            # C: ACT copy + Pool TT -> bf16
            opts.append(("C", {"act": (OVH_ACT + fd) / RATE_ACT,
                               "pool": (OVH_POOL + fd) / RATE_POOL,
                               "dma": byt16}))
        best = None
        for name, cost in opts:
            mk = max(t[k] + cost.get(k, 0.0) for k in t)
            if best is None or mk < best[0]:
                best = (mk, name, cost)
        out.append(best[1])
        for k, v in best[2].items():
            t[k] += v
    return out


refresh_layout()


def _split_bf16(a):
    hi = a.astype(_bf16)
    lo = (a - hi.astype(np.float32)).astype(_bf16)
    return hi, lo


def ap2d(sliced, dims):
    """Copy of AP `sliced` with its free dims replaced by [step, count]."""
    c = sliced.copy()
    v = c.ap
    part = [list(v[0])]
    while len(v) > 0:
        v.pop()
    for d in part + [list(x) for x in dims]:
        v.append(d)
    c.ap = v
    return c


def _build_nc():
    nc = bacc.Bacc("TRN2", target_bir_lowering=False, debug=False,
                   num_devices=NCORES)
    f32 = mybir.dt.float32
    bf16 = mybir.dt.bfloat16

    wdims = _SW if PE_SLOTS else _GW
    nxt = NSLOT if PE_SLOTS else 4
    xtc = _SIDX * 128 if PE_SLOTS else 128 * 128
    xt_d = nc.dram_tensor("xt4", (nxt, 4, xtc), bf16,
                          kind="ExternalInput").ap()
    wp_d = [nc.dram_tensor(f"wp{g}", (4, wdims[g] + _WPAD), bf16,
                           kind="ExternalInput").ap() for g in range(nxt)]
    xse_d = nc.dram_tensor("xse", (BS, NF + _XPAD), bf16,
                           kind="ExternalInput").ap()
    xso_d = nc.dram_tensor("xso", (BS, NF + _XPAD), bf16,
                           kind="ExternalInput").ap()
    out8_d = nc.dram_tensor("out8", (BS, PPAD), mybir.dt.int8,
                            kind="ExternalOutput").ap()
    out16_d = nc.dram_tensor("out16", (BS, PPAD), bf16,
                             kind="ExternalOutput").ap()

    paths = _PATHS

    with TileContext(nc) as tc:
        with tc.tile_pool(name="sb", bufs=1) as sb, \
             tc.tile_pool(name="stg", bufs=STAGE_BUFS) as stg, \
             tc.tile_pool(name="stg8", bufs=STAGE_BUFS) as stg8, \
             tc.tile_pool(name="tmp", bufs=TMP_BUFS) as tmpp, \
             tc.tile_pool(name="ps", bufs=PSUM_BUFS, space="PSUM") as ps:

            xse = sb.tile([128, NF + _XPAD], bf16, tag="xse")
            xso = sb.tile([128, NF + _XPAD], bf16, tag="xso")
            nc.sync.dma_start(out=xse[:], in_=xse_d[:])
            nc.sync.dma_start(out=xso[:], in_=xso_d[:])

            xt = sb.tile([128, xtc], bf16, tag="xt")
            wp = sb.tile([128, max(wdims) + _WPAD], bf16, tag="wp")
            for g in range(nxt):
                nc.sync.dma_start(out=xt[32 * g:32 * g + 4, :], in_=xt_d[g])
                nc.sync.dma_start(out=wp[32 * g:32 * g + 4, 0:wdims[g] + _WPAD],
                                  in_=wp_d[g][:])

            if PE_SLOTS:
                def lhs(j1):
                    s = _J1SLOT[j1]
                    i = _SIDXMAP[j1]
                    return xt[32 * s:32 * s + 4, i * 128:(i + 1) * 128]

                def rhs(j1, n):
                    s = _J1SLOT[j1]
                    lo = _SOFF[j1]
                    return wp[32 * s:32 * s + 4, lo:lo + n]

                def tpos(j1):
                    return (32 * _J1SLOT[j1], 0)
            else:
                def lhs(j1):
                    g = j1 // 128
                    r = j1 - 128 * g
                    return xt[32 * g:32 * g + 4, r * 128:(r + 1) * 128]

                def rhs(j1, n):
                    g = j1 // 128
                    lo = _off(j1) - _GOFF[g]
                    return wp[32 * g:32 * g + 4, lo:lo + n]

                def tpos(j1):
                    return (32 * (j1 // 128), 0)

            for _rep in range(REPS):
                main_pass(nc, stg, stg8, tmpp, ps, xse, xso, lhs, rhs, tpos,
                          out8_d, out16_d, paths, f32, bf16)

    nc.compile()
    return nc


def main_pass(nc, stg, stg8, tmpp, ps, xse, xso, lhs, rhs, tpos, out8_d,
              out16_d, paths, f32, bf16):
    i8 = mybir.dt.int8
    out_flat = {"bf": out16_d.rearrange("a b -> (a b)"),
                "i8": out8_d.rearrange("a b -> (a b)")}
    pool = {"bf": stg, "i8": stg8}
    chw = {"bf": CH, "i8": CH8}
    cdt = {"bf": bf16, "i8": i8}

    if MODE == "dma":
        stage = stg.tile([128, CH], bf16, tag="stage")
        nc.vector.memset(stage[:], 0.0)
        pos = 0
        while pos < PPAD:
            w = min(CH, PPAD - pos)
            dst = out_flat["bf"][pos * 128:pos * 128 + 128 * w]
            dst = dst.rearrange("(p f) -> p f", p=128)
            nc.sync.dma_start(out=dst, in_=stage[:, 0:w])
            pos += w
        return

    st_state = {}
    for st in ("bf", "i8"):
        st_state[st] = {"cur": 0, "base": 0, "nflush": 0, "stage": None}

    def new_stage(st):
        stage = pool[st].tile([128, chw[st]], cdt[st], tag="stage_" + st)
        st_state[st]["stage"] = stage

    def flush(st):
        s = st_state[st]
        if s["cur"] == 0 or MODE == "pe":
            s["cur"] = 0
            return
        if s["nflush"] % FLUSH_EVERY == 0:
            dst = out_flat[st][s["base"] * 128:(s["base"] + s["cur"]) * 128]
            dst = dst.rearrange("(p f) -> p f", p=128)
            eng = getattr(nc, DMA_ENGINES[s["nflush"] % len(DMA_ENGINES)])
            eng.dma_start(out=dst, in_=s["stage"][:, 0:s["cur"]])
        s["nflush"] += 1
        s["base"] += s["cur"]
        s["cur"] = 0
        new_stage(st)

    if MODE != "pe":
        new_stage("bf")
        new_stage("i8")

    for gi, g in enumerate(_GROUPS):
        j1, M, m_b, n_pad, ns = g["j1"], g["M"], g["m_b"], g["n_pad"], g["ns"]
        width = M * n_pad
        path = paths[gi] if MODE == "full" else ("A" if MODE == "dve" else "B")
        st = "i8" if (path == "A" and INT8_OUT) else "bf"
        if MODE != "pe" and st_state[st]["cur"] + width > chw[st]:
            flush(st)
        cur = st_state[st]["cur"] if MODE != "pe" else 0
        stage = st_state[st]["stage"] if MODE != "pe" else None

        psum = ps.tile([128, 2048], f32, tag="psum")
        if MODE in ("full", "pe"):
            for k in range(M):
                nk = ns[k]
                if m_b == 1:
                    pcol = 512 * k
                else:
                    # bank = k % 4: consecutive blocks (which may execute
                    # concurrently in different PE row groups) never share a
                    # PSUM bank -- same-bank writes from different row
                    # groups crash the device.
                    pcol = 512 * (k % 4) + (k // 4) * n_pad
                nc.tensor.matmul(psum[:, pcol:pcol + nk],
                                 lhs(j1 + k), rhs(j1 + k, nk),
                                 start=True, stop=True,
                                 tile_position=tpos(j1 + k))
        elif MODE in ("dve", "act"):
            n0 = ns[0]
            nc.tensor.matmul(psum[:, 0:n0], lhs(j1), rhs(j1, n0),
                             start=True, stop=True,
                             tile_position=tpos(j1))
        if MODE == "pe":
            continue

        if m_b == 1:
            p_dims = [[512, M], [1, n_pad]]
            x_dims = [[1, M], [1, n_pad]]
        else:
            # block k sits at psum col 512*(k%4) + (k//4)*n_pad; iterate
            # slot-major/bank-minor so coverage order is ascending k
            p_dims = [[n_pad, m_b], [512, 4], [1, n_pad]]
            x_dims = [[4, m_b], [1, 4], [1, n_pad]]
        t_dims = [[n_pad, M], [1, n_pad]]

        if path == "A":
            out_ap = ap2d(stage[:, cur:cur + 1], t_dims)
            in0_ap = ap2d(psum[:, 0:1], p_dims)
            in1_ap = ap2d(xse[:, j1 + 1:j1 + 2], x_dims)
            nc.vector.tensor_mul(out=out_ap, in0=in0_ap, in1=in1_ap)
        else:
            tmp = tmpp.tile([128, 2048], bf16, tag="tmp")
            t_ap = ap2d(tmp[:, 0:1], t_dims)
            p_ap = ap2d(psum[:, 0:1], p_dims)
            nc.scalar.copy(out=t_ap, in_=p_ap)
            if path == "C":
                # Pool tensor_mul, bf16 in/out, no parity constraint
                o_ap = ap2d(stage[:, cur:cur + 1], t_dims)
                i0_ap = ap2d(tmp[:, 0:1], t_dims)
                i1_ap = ap2d(xse[:, j1 + 1:j1 + 2], [[1, M], [1, n_pad]])
                nc.gpsimd.tensor_mul(out=o_ap, in0=i0_ap, in1=i1_ap)
            else:
                # split by block parity so every innermost run starts
                # 4B-aligned (even x-col via xse, odd via shifted xso)
                for par in range(min(2, M)):
                    col = j1 + 1 + par
                    src, scol = (xse, col) if col % 2 == 0 else (xso, col - 1)
                    cnt = (M - par + 1) // 2
                    s_dims = [[2 * n_pad, cnt], [1, n_pad]]
                    i1_dims = [[2, cnt], [1, n_pad]]
                    o_ap = ap2d(
                        stage[:, cur + par * n_pad:cur + par * n_pad + 1],
                        s_dims)
                    i0_ap = ap2d(tmp[:, par * n_pad:par * n_pad + 1], s_dims)
                    i1_ap = ap2d(src[:, scol:scol + 1], i1_dims)
                    nc.vector.tensor_mul(out=o_ap, in0=i0_ap, in1=i1_ap)
        st_state[st]["cur"] += width
        if st_state[st]["cur"] >= chw[st]:
            flush(st)
    if MODE != "pe":
        flush("bf")
        flush("i8")


_NC_CACHE = None
_NC_CACHE_KEY = None


_LAST_S = 1.0


def _host_inputs(x, weight):
    global _LAST_S
    i1, i2 = np.triu_indices(NF, k=1)
    wdot = np.einsum("pk,pk->p", weight[i1].astype(np.float64),
                     weight[i2].astype(np.float64)).astype(np.float32)
    s = 1.0
    if INT8_OUT:
        # exact absmax of the output via upper-bound pruning, for the int8
        # quantization scale (folded into wdot on the int8 groups)
        mx = np.abs(x).max(axis=0)
        ub = np.abs(wdot) * mx[i1] * mx[i2]
        order = np.argsort(-ub)
        best = 0.0
        for off in range(0, P, 4096):
            idx = order[off:off + 4096]
            if ub[idx[0]] <= best:
                break
            cols = np.abs(x[:, i1[idx]] * x[:, i2[idx]]
                          * wdot[None, idx]).max(axis=0)
            best = max(best, float(cols.max()))
        s = 1.02 * best / 127.0
    _LAST_S = s
    fac = np.ones(P, np.float32)
    for g, st in zip(_GROUPS, _STREAMS):
        if st == "i8":
            fac[_off(g["j1"]):_off(g["j1"] + g["M"])] = 1.0 / s
    wh, wl = _split_bf16(wdot * fac)
    wp_in = {}
    if PE_SLOTS:
        for s in range(NSLOT):
            arr = np.zeros((4, _SW[s] + _WPAD), dtype=_bf16)
            cat_h = np.concatenate(
                [wh[_off(j1):_off(j1) + NF - 1 - j1] for j1 in _SFEAT[s]])
            cat_l = np.concatenate(
                [wl[_off(j1):_off(j1) + NF - 1 - j1] for j1 in _SFEAT[s]])
            arr[0, 0:_SW[s]] = cat_h
            arr[1, 0:_SW[s]] = cat_h
            arr[2, 0:_SW[s]] = cat_l
            arr[3, 0:_SW[s]] = cat_l
            wp_in[f"wp{s}"] = arr
    else:
        for g in range(4):
            arr = np.zeros((4, _GW[g] + _WPAD), dtype=_bf16)
            sl = slice(_GOFF[g], _GOFF[g + 1])
            arr[0, 0:_GW[g]] = wh[sl]
            arr[1, 0:_GW[g]] = wh[sl]
            arr[2, 0:_GW[g]] = wl[sl]
            arr[3, 0:_GW[g]] = wl[sl]
            wp_in[f"wp{g}"] = arr

    in_maps = []
    for c in range(NCORES):
        xc = x[c * BS:(c + 1) * BS]           # [128, 512] fp32
        xct = np.ascontiguousarray(xc.T)      # [512, 128]
        xh, xl = _split_bf16(xct)
        if PE_SLOTS:
            xt4 = np.zeros((NSLOT, 4, _SIDX * 128), dtype=_bf16)
            for s in range(NSLOT):
                fh = np.ascontiguousarray(xh[_SFEAT[s]]).reshape(-1)
                fl = np.ascontiguousarray(xl[_SFEAT[s]]).reshape(-1)
                xt4[s, 0, 0:fh.size] = fh
                xt4[s, 1, 0:fl.size] = fl
                xt4[s, 2, 0:fh.size] = fh
                xt4[s, 3, 0:fl.size] = fl
        else:
            xt4 = np.empty((4, 4, 128 * 128), dtype=_bf16)
            for g in range(4):
                fh = xh[128 * g:128 * (g + 1)].reshape(-1)
                fl = xl[128 * g:128 * (g + 1)].reshape(-1)
                xt4[g, 0] = fh
                xt4[g, 1] = fl
                xt4[g, 2] = fh
                xt4[g, 3] = fl
        xb = np.zeros((BS, NF + _XPAD), dtype=_bf16)
        xb[:, 0:NF] = xc.astype(_bf16)
        xo = np.zeros((BS, NF + _XPAD), dtype=_bf16)
        xo[:, 0:NF - 1] = xb[:, 1:NF]
        m = {"xt4": xt4, "xse": xb, "xso": xo}
        m.update(wp_in)
        in_maps.append(m)
    return in_maps


def _chunk_widths():
    """Mirror of main_pass flush logic: widths of the DMA'd chunks."""
    widths = []
    cur = 0
    for g in _GROUPS:
        width = g["M"] * g["n_pad"]
        if cur + width > CH:
            widths.append(cur)
            cur = 0
        cur += width
        if cur >= CH:
            widths.append(cur)
            cur = 0
    if cur:
        widths.append(cur)
    return widths


_IDX_CACHE = None


def _pad_index():
    global _IDX_CACHE
    if _IDX_CACHE is None:
        idx = np.empty(P, dtype=np.int64)
        for g in _GROUPS:
            j1, M, n_pad, ns, pp = g["j1"], g["M"], g["n_pad"], g["ns"], g["pp"]
            for k in range(M):
                ot = _off(j1 + k)
                idx[ot:ot + ns[k]] = pp + k * n_pad + np.arange(ns[k])
        _IDX_CACHE = idx
    return _IDX_CACHE


def kernel(x, weight):
    global _NC_CACHE, _NC_CACHE_KEY, LAST_RESULT, _last_in_maps
    x = np.ascontiguousarray(x, dtype=np.float32)
    weight = np.ascontiguousarray(weight, dtype=np.float32)
    assert x.shape == (B, NF) and weight.shape == (NF, K)

    in_maps = _host_inputs(x, weight)
    _last_in_maps = in_maps

    key = (MODE, CH, CH8, FLUSH_EVERY, REPS, PE_SLOTS, PACK_BANKS,
           SPLIT_FRAC_A, INT8_OUT, USE_POOL, STAGE_BUFS, TMP_BUFS,
           RATE_POOL, DMA_ENGINES)
    if _NC_CACHE is None or _NC_CACHE_KEY != key:
        _NC_CACHE = _build_nc()
        _NC_CACHE_KEY = key
    nc = _NC_CACHE

    res = bass_utils.run_bass_kernel_spmd(nc, in_maps,
                                          core_ids=list(range(NCORES)),
                                          trace=TRACE)
    LAST_RESULT = {"exec_time_ns": res.exec_time_ns,
                   "trace": res.instructions_and_trace}

    s = _LAST_S
    out = np.empty((B, P), np.float32)
    for c, r in enumerate(res.results):
        full = out[c * BS:(c + 1) * BS]
        for st, name in (("bf", "out16"), ("i8", "out8")):
            blob = _BLOB[st]
            tot = blob["total"]
            if tot == 0:
                continue
            raw = np.asarray(r[name]).reshape(-1)[:128 * tot]
            if st == "bf":
                raw = raw.view(np.uint16)
            pad = np.empty((BS, tot), raw.dtype)
            pos = 0
            for w in blob["widths"]:
                pad[:, pos:pos + w] = \
                    raw[128 * pos:128 * (pos + w)].reshape(BS, w)
                pos += w
            gat = pad[:, blob["idx_s"]]
            if st == "bf":
                full[:, blob["idx_t"]] = \
                    (gat.astype(np.uint32) << 16).view(np.float32)
            else:
                full[:, blob["idx_t"]] = gat.astype(np.float32) * s
    return out
